# revision 1
# baseline (speedup 1.0000x reference)
"""Trainium2 Bass kernel for ContrastiveResNetGCN (pipelined, bf16).

Reference computation (N=8192, D=512, P=128, H=128):
    x_proj = relu(x1 @ W1) @ W2                       [N,P]
    A      = cos_sim(x_proj)  (eps clamp never binds) [N,N]
    out    = (A @ (x1 @ gc_w)) / N + gc_b             [N,H]

Rank-P factorization (A = g g^T, g = x_proj/||x_proj||_row) removes both
[N,N] matmuls.  Per core (rows sharded 8 x 1024):
    u    = x_proj                    [R,P]
    winv = 1/||u||_row ; v = h * winv / N
    Mp   = u^T @ v                   [P,H]  partial -> AllReduce(add)
    out  = (u @ M) * winv + gc_b

Perf design vs the serial baseline (65.4 us/exec):
  * AllReduce into a NON-Shared DRAM buffer.  DMA reads from Shared
    address space measure ~140 us for 64KB on this fabric; non-Shared
    read-back + AllReduce pipeline at ~3 us.
  * Software pipelining: the tail of rep i (CC read-back + final
    matmuls + store) is emitted after the head of rep i+2, with
    3-deep buffer rotation, so the collective latency fully hides
    behind the next reps' compute.  For nreps=1 (the correctness
    path) order degenerates to head;tail — still correct.
  * bf16 matmul path everywhere (PSUM accumulation stays f32):
    wide matmuls run at the same 1 cyc/row as f32r, PE transposes are
    2x faster, and the 128-wide M/out matmuls are 4x faster than f32.
  * h = x1 @ gc_w computed directly in node-major layout (lhsT =
    x1T 128-col blocks), skipping a PSUM->SBUF->PSUM transpose trip.

`nreps` unrolls the computation inside one NEFF; exec_ns is the slope
of wall(nreps), amortizing the (tens of ms) axon dispatch overhead.
"""

import os
import sys

import numpy as np

for _p in ("/opt/trn_rl_repo", "/opt/pypackages"):
    if os.path.isdir(_p) and _p not in sys.path:
        sys.path.append(_p)

import concourse.bass as bass
import concourse.mybir as mybir
from concourse import bacc
import concourse.tile as tile
from concourse.bass_utils import run_bass_kernel_spmd
from concourse.masks import make_identity

F32 = mybir.dt.float32
BF16 = mybir.dt.bfloat16
AF = mybir.ActivationFunctionType
ALU = mybir.AluOpType

N, D, P, H = 8192, 512, 128, 128
NCORES = 8
R = N // NCORES          # rows per core (1024)
NT = R // 128            # node tiles per core (8)
KD = D // 128            # contraction tiles over D (4)
NG = R // 512            # node groups of 512 (2)
INV_N = 1.0 / N
DEFER = int(os.environ.get("KERNEL_DEFER", "4"))  # tail of rep i emitted after head of rep i+DEFER


def sl(i, w=128):
    return slice(i * w, (i + 1) * w)


def build_bass(nreps: int = 1, no_cc: bool = False):
    nc = bacc.Bacc("TRN2", target_bir_lowering=False, num_devices=NCORES)

    x1 = nc.dram_tensor("x1", [R, D], F32, kind="ExternalInput")
    w1 = nc.dram_tensor("w1", [D, D], F32, kind="ExternalInput")
    w2 = nc.dram_tensor("w2", [D, P], F32, kind="ExternalInput")
    gcw = nc.dram_tensor("gcw", [D, H], F32, kind="ExternalInput")
    gcb = nc.dram_tensor("gcb", [H], F32, kind="ExternalInput")
    out = nc.dram_tensor("out", [R, H], F32, kind="ExternalOutput")

    with tile.TileContext(nc) as tc:
        with (
            tc.tile_pool(name="cpool", bufs=1) as cpool,
            tc.tile_pool(name="xload", bufs=8) as xload,
            tc.tile_pool(name="big", bufs=2) as bpool,
            tc.tile_pool(name="ut3", bufs=DEFER + 1) as ut3,
            tc.tile_pool(name="small", bufs=2) as spool,
            tc.tile_pool(name="w3", bufs=DEFER + 1) as w3,
            tc.tile_pool(name="opool", bufs=3) as opool,
            tc.tile_pool(name="ptr", bufs=2, space="PSUM") as ptr,
            tc.tile_pool(name="pmm", bufs=3, space="PSUM") as pmm,
            tc.tile_pool(name="pM", bufs=1, space="PSUM") as pMp,
            tc.tile_pool(name="pfin", bufs=2, space="PSUM") as pfin,
            tc.tile_pool(name="dram", bufs=DEFER + 1, space="DRAM") as dram,
        ):
            # ---- constants / weights (loaded once, converted to bf16) ----
            identb = cpool.tile([128, 128], BF16, name="identb")
            make_identity(nc, identb)
            identf = cpool.tile([128, 128], F32, name="identf")
            make_identity(nc, identf)

            w1b, w2b, gcwb = [], [], []
            for kd in range(KD):
                t = xload.tile([128, D], F32, name="wload", tag="wload")
                nc.sync.dma_start(out=t, in_=w1[sl(kd), :])
                b = cpool.tile([128, D], BF16, name=f"w1b_{kd}")
                nc.vector.tensor_copy(b, t)
                w1b.append(b)
            for kd in range(KD):
                t = xload.tile([128, P], F32, name="wload", tag="wload")
                nc.sync.dma_start(out=t, in_=w2[sl(kd), :])
                b = cpool.tile([128, P], BF16, name=f"w2b_{kd}")
                nc.vector.tensor_copy(b, t)
                w2b.append(b)
            for kd in range(KD):
                t = xload.tile([128, H], F32, name="wload", tag="wload")
                nc.sync.dma_start(out=t, in_=gcw[sl(kd), :])
                b = cpool.tile([128, H], BF16, name=f"gcwb_{kd}")
                nc.vector.tensor_copy(b, t)
                gcwb.append(b)

            b_row = cpool.tile([1, H], F32, name="b_row")
            nc.sync.dma_start(out=b_row, in_=gcb[None, :])
            ones1 = cpool.tile([1, 128], F32, name="ones1")
            nc.vector.memset(ones1, 1.0)
            # bb[p, q] = gcb[q] for all p (partition-broadcast via K=1 matmul)
            bb = cpool.tile([128, H], F32, name="bb")
            pbb = pfin.tile([128, H], F32, name="pbb", tag="pp")
            nc.tensor.matmul(pbb, lhsT=ones1, rhs=b_row, start=True, stop=True)
            nc.scalar.copy(bb, pbb)

            consts = (identb, identf, w1b, w2b, gcwb, bb)
            pools = (xload, bpool, ut3, spool, w3, opool,
                     ptr, pmm, pMp, pfin, dram)

            inflight = {}
            for rep in range(nreps):
                inflight[rep] = _emit_head(nc, x1, pools, consts, no_cc)
                t = rep - DEFER
                if t >= 0:
                    _emit_tail(nc, out, pools, consts, inflight.pop(t), no_cc)
            for t in sorted(inflight):
                _emit_tail(nc, out, pools, consts, inflight.pop(t), no_cc)

    nc.compile()
    return nc


def _emit_head(nc, x1, pools, consts, no_cc):
    (xload, bpool, ut3, spool, w3, opool,
     ptr, pmm, pMp, pfin, dram) = pools
    (identb, identf, w1b, w2b, gcwb, bb) = consts

    x1T = [bpool.tile([128, R], BF16, name=f"x1T_{kd}", tag=f"x1T_{kd}")
           for kd in range(KD)]
    Bt = [bpool.tile([128, R], BF16, name=f"Bt_{mf}", tag=f"Bt_{mf}")
          for mf in range(KD)]
    ut = ut3.tile([128, R], BF16, name="ut", tag="ut")
    u4 = [bpool.tile([128, 512], BF16, name=f"u4_{g}", tag=f"u4_{g}")
          for g in range(NG)]
    v4 = [bpool.tile([128, 512], BF16, name=f"v4_{g}", tag=f"v4_{g}")
          for g in range(NG)]
    winv = w3.tile([128, NT], F32, name="winv", tag="winv")
    ssq = spool.tile([128, NT], F32, name="ssq", tag="ssq")

    # ---- stage 1: load x1 (8 DMAs spread across queues), transpose -----
    for g in range(NG):
        gs = sl(g, 512)
        xrs = []
        for j in range(4):
            xr = xload.tile([128, D], F32, name="xr", tag="xr")
            nc.sync.dma_start(out=xr, in_=x1[sl(4 * g + j), :])
            xrs.append(xr)
        for kd in range(KD):
            ptx = ptr.tile([128, 512], F32, name="ptx", tag="tr")
            for j in range(4):
                nc.tensor.transpose(ptx[:, sl(j)], xrs[j][:, sl(kd)], identf)
            nc.vector.tensor_copy(x1T[kd][:, gs], ptx)

    # ---- stage 2: projector chains -------------------------------------
    for g in range(NG):
        gs = sl(g, 512)
        for mf in range(KD):
            pb = pmm.tile([128, 512], F32, name="pb", tag="mm")
            for kd in range(KD):
                nc.tensor.matmul(
                    pb,
                    lhsT=w1b[kd][:, sl(mf)],
                    rhs=x1T[kd][:, gs],
                    start=(kd == 0),
                    stop=(kd == KD - 1),
                )
            nc.scalar.activation(Bt[mf][:, gs], pb, AF.Relu)
    for g in range(NG):
        gs = sl(g, 512)
        pu = pmm.tile([128, 512], F32, name="pu", tag="mm")
        for mf in range(KD):
            nc.tensor.matmul(
                pu,
                lhsT=w2b[mf],
                rhs=Bt[mf][:, gs],
                start=(mf == 0),
                stop=(mf == KD - 1),
            )
        nc.vector.tensor_copy(ut[:, gs], pu)

    # ---- stage 3: u -> node-major + row norms --------------------------
    for g in range(NG):
        ptu = ptr.tile([128, 512], BF16, name="ptu", tag="tr")
        for j in range(4):
            nc.tensor.transpose(ptu[:, sl(j)], ut[:, sl(4 * g + j)], identb)
        nc.vector.tensor_copy(u4[g], ptu)
        for j in range(4):
            m = 4 * g + j
            sq = spool.tile([128, 128], F32, name="sq", tag="sq")
            nc.scalar.activation(sq, ptu[:, sl(j)], AF.Square,
                                 accum_out=ssq[:, m:m + 1])
    wv = spool.tile([128, NT], F32, name="wv", tag="wv")
    nc.scalar.activation(wv, ssq, AF.Sqrt)
    nc.vector.reciprocal(winv, wv)

    # ---- stage 4: h = x1 @ gcw in node-major; v = h * winv / N ---------
    for g in range(NG):
        ph = pmm.tile([128, 512], F32, name="ph", tag="mm")
        for j in range(4):
            m = 4 * g + j
            for kd in range(KD):
                nc.tensor.matmul(
                    ph[:, sl(j)],
                    lhsT=x1T[kd][:, sl(m)],
                    rhs=gcwb[kd],
                    start=(kd == 0),
                    stop=(kd == KD - 1),
                )
        for j in range(4):
            m = 4 * g + j
            nc.vector.tensor_scalar(
                v4[g][:, sl(j)], ph[:, sl(j)], winv[:, m:m + 1], INV_N,
                op0=ALU.mult, op1=ALU.mult,
            )

    # ---- stage 5: M partial + AllReduce kickoff ------------------------
    pM = pMp.tile([128, H], F32, name="pM", tag="pM")
    for m in range(NT):
        g, j = divmod(m, 4)
        nc.tensor.matmul(
            pM,
            lhsT=u4[g][:, sl(j)],
            rhs=v4[g][:, sl(j)],
            start=(m == 0),
            stop=(m == NT - 1),
        )
    Msb = spool.tile([128, H], F32, name="Msb", tag="Msb")
    nc.scalar.copy(Msb, pM)
    cc_out = None
    if not no_cc:
        cc_in = dram.tile([128, H], F32, name="cc_in", tag="cc_in")
        cc_out = dram.tile([128, H], F32, name="cc_out", tag="cc_out")
        nc.sync.dma_start(out=cc_in, in_=Msb)
        nc.gpsimd.collective_compute(
            "AllReduce",
            ALU.add,
            replica_groups=[list(range(NCORES))],
            ins=[cc_in[:, :]],
            outs=[cc_out[:, :]],
        )
    return (cc_out, Msb, ut, winv)


def _emit_tail(nc, out, pools, consts, state, no_cc):
    (xload, bpool, ut3, spool, w3, opool,
     ptr, pmm, pMp, pfin, dram) = pools
    (identb, identf, w1b, w2b, gcwb, bb) = consts
    (cc_out, Msb, ut, winv) = state

    Mb = spool.tile([128, H], BF16, name="Mb", tag="Mb")
    if no_cc:  # timing-only variant: skip the collective (math wrong)
        nc.vector.tensor_copy(Mb, Msb)
    else:
        Mred = spool.tile([128, H], F32, name="Mred", tag="Mred")
        nc.sync.dma_start(out=Mred, in_=cc_out[:, :])
        nc.vector.tensor_copy(Mb, Mred)

    # ---- out = (u @ M) * winv + bb -------------------------------------
    for m in range(NT):
        pp = pfin.tile([128, H], F32, name="pp", tag="pp")
        nc.tensor.matmul(pp, lhsT=ut[:, sl(m)], rhs=Mb, start=True, stop=True)
        ob = opool.tile([128, H], F32, name="ob", tag="ob")
        nc.vector.scalar_tensor_tensor(
            ob, pp, winv[:, m:m + 1], bb, op0=ALU.mult, op1=ALU.add
        )
        nc.sync.dma_start(out=out[sl(m), :], in_=ob)


_NCS = {}
LAST_RESULTS = None
_RUNNERS = {}


def _get_nc(nreps: int = 1):
    key = (nreps, os.environ.get("KERNEL_NO_CC") == "1")
    if key not in _NCS:
        _NCS[key] = build_bass(nreps, no_cc=key[1])
    return _NCS[key]


def _split_in_maps(inputs):
    x1 = np.ascontiguousarray(np.asarray(inputs["x1"], dtype=np.float32))
    w1 = np.ascontiguousarray(np.asarray(inputs["proj_w1"], dtype=np.float32))
    w2 = np.ascontiguousarray(np.asarray(inputs["proj_w2"], dtype=np.float32))
    gcw = np.ascontiguousarray(np.asarray(inputs["gc_w"], dtype=np.float32))
    gcb = np.ascontiguousarray(np.asarray(inputs["gc_b"], dtype=np.float32))
    return [
        {
            "x1": np.ascontiguousarray(x1[c * R:(c + 1) * R]),
            "w1": w1,
            "w2": w2,
            "gcw": gcw,
            "gcb": gcb,
        }
        for c in range(NCORES)
    ]


def kernel(**inputs) -> np.ndarray:
    global LAST_RESULTS
    res = run_bass_kernel_spmd(
        _get_nc(1), _split_in_maps(inputs), core_ids=list(range(NCORES))
    )
    LAST_RESULTS = res
    return np.concatenate([res.results[c]["out"] for c in range(NCORES)], axis=0)


# ---------------------------------------------------------------------------
# Timing path: the nreps-unrolled NEFF amortizes the (tens of ms) axon
# dispatch overhead; per-exec time = slope between two nreps points.
# ---------------------------------------------------------------------------

def _make_runner(nreps: int):
    if nreps in _RUNNERS:
        return _RUNNERS[nreps]
    import jax
    import concourse.mybir as mybir_
    from concourse.bass2jax import (
        _bass_exec_p,
        install_neuronx_cc_hook,
        partition_id_tensor,
    )
    from jax.experimental.shard_map import shard_map
    from jax.sharding import Mesh, PartitionSpec

    nc = _get_nc(nreps)
    install_neuronx_cc_hook()
    partition_name = (
        nc.partition_id_tensor.name if nc.partition_id_tensor else None
    )

    in_names, out_names, out_avals = [], [], []
    for alloc in nc.m.functions[0].allocations:
        if not isinstance(alloc, mybir_.MemoryLocationSet):
            continue
        name = alloc.memorylocations[0].name
        if alloc.kind == "ExternalInput":
            if name != partition_name:
                in_names.append(name)
        elif alloc.kind == "ExternalOutput":
            out_names.append(name)
            out_avals.append(
                jax.core.ShapedArray(
                    tuple(alloc.tensor_shape), mybir_.dt.np(alloc.dtype)
                )
            )
    n_params = len(in_names)
    all_names = in_names + out_names
    if partition_name is not None:
        all_names = all_names + [partition_name]

    def _body(*args):
        operands = list(args)
        if partition_name is not None:
            operands.append(partition_id_tensor())
        outs = _bass_exec_p.bind(
            *operands,
            out_avals=tuple(out_avals),
            in_names=tuple(all_names),
            out_names=tuple(out_names),
            lowering_input_output_aliases=(),
            sim_require_finite=True,
            sim_require_nnan=True,
            nc=nc,
        )
        return tuple(outs)

    devices = jax.devices()[:NCORES]
    mesh = Mesh(np.asarray(devices), ("core",))
    nin = n_params + len(out_names)
    sharded = jax.jit(
        shard_map(
            _body,
            mesh=mesh,
            in_specs=(PartitionSpec("core"),) * nin,
            out_specs=(PartitionSpec("core"),) * len(out_names),
            check_rep=False,
        ),
        keep_unused=True,
    )
    meta = (in_names, out_names, out_avals, n_params)
    _RUNNERS[nreps] = (sharded, meta)
    return _RUNNERS[nreps]


def _prep_args(inputs, nreps: int):
    import jax

    sharded, meta = _make_runner(nreps)
    in_names, out_names, out_avals, n_params = meta
    in_maps = _split_in_maps(inputs)
    concat_in = [
        np.concatenate([np.asarray(in_maps[c][n]) for c in range(NCORES)], axis=0)
        for n in in_names
    ]
    concat_zeros = [
        np.zeros((NCORES * a.shape[0], *a.shape[1:]), a.dtype) for a in out_avals
    ]
    args = [jax.device_put(a) for a in concat_in + concat_zeros]
    for a in args:
        a.block_until_ready()
    return sharded, args


def _timed_call(sharded, args):
    import time

    t0 = time.perf_counter()
    outs = sharded(*args)
    for o in outs:
        o.block_until_ready()
    return time.perf_counter() - t0, outs


def run_repeated(inputs, nreps: int, iters: int = 6):
    """Run the nreps-unrolled NEFF; returns (out_core0, min_wall_seconds)."""
    sharded, args = _prep_args(inputs, nreps)
    _timed_call(sharded, args)  # warmup/compile
    times = []
    outs = None
    for _ in range(iters):
        t, outs = _timed_call(sharded, args)
        times.append(t)
    return np.asarray(outs[0]), min(times)


def measure_exec_ns(inputs, k1=8, k2=104, rounds=25):
    """Amortized per-exec device time via interleaved two-point slope.

    Axon dispatch overhead drifts by tens of ms over minutes, so the k1
    and k2 NEFFs are compiled+warmed FIRST, then timed strictly
    interleaved so both sample the same overhead distribution; the
    min-over-rounds of each cancels the (additive, positive) noise.
    """
    import statistics

    s1, a1 = _prep_args(inputs, k1)
    s2, a2 = _prep_args(inputs, k2)
    for _ in range(3):  # warm both (compile, caches, power state)
        _timed_call(s1, a1)
        _timed_call(s2, a2)
    t1s, t2s = [], []
    outs2 = None
    for _ in range(rounds):
        t1, _o = _timed_call(s1, a1)
        t2, outs2 = _timed_call(s2, a2)
        t1s.append(t1)
        t2s.append(t2)
    dk = k2 - k1
    slope_min = (min(t2s) - min(t1s)) / dk
    pair = sorted((b - a) / dk for a, b in zip(t1s, t2s))
    slope_pairmed = statistics.median(pair)
    print(
        f"timing diag: min-min={slope_min*1e9:.0f}ns "
        f"pair-med={slope_pairmed*1e9:.0f}ns "
        f"pair-q25={pair[len(pair)//4]*1e9:.0f}ns "
        f"pair-q75={pair[3*len(pair)//4]*1e9:.0f}ns "
        f"t1(min/med)={min(t1s)*1e3:.2f}/{statistics.median(t1s)*1e3:.2f}ms "
        f"t2(min/med)={min(t2s)*1e3:.2f}/{statistics.median(t2s)*1e3:.2f}ms"
    )
    per_exec = slope_min
    return per_exec * 1e9, np.asarray(outs2[0]), min(t1s), min(t2s)



# revision 4
# speedup vs baseline: 1.8016x; 1.8016x over previous
"""Trainium2 Bass kernel for ContrastiveResNetGCN (pipelined, bf16, AllGather).

Reference computation (N=8192, D=512, P=128, H=128):
    x_proj = relu(x1 @ W1) @ W2                       [N,P]
    A      = cos_sim(x_proj)  (eps clamp never binds) [N,N]
    out    = (A @ (x1 @ gc_w)) / N + gc_b             [N,H]

Rank-P factorization (A = g g^T, g = x_proj/||x_proj||_row) removes both
[N,N] matmuls.  Per core (rows sharded 8 x 1024):
    u    = x_proj                    [R,P]
    winv = 1/||u||_row
    Q    = (u*winv/N)^T @ x1         [P,D]  (contraction over local nodes)
    Mp   = Q @ gc_w                  [P,H]  partial -> AllGather, local sum
    out  = (u @ M) * winv + gc_b

Perf design (from perfetto traces of the previous AllReduce version):
  * v1 (AllReduce [128,128]f32/rep) was collective-THROUGHPUT bound:
    cc_op_active was 92% of the span at nreps=104 and the mesh
    AllReduces ran back-to-back at ~27-29us each => ~29.5us/rep.
    AllReduce = ReduceScatter+AllGather; the reduce pass doubles the
    M2S descriptor traffic.  AllGather of the 8 partials (64KB in,
    512KB out) + a 7-op DVE tree-sum is ~2x cheaper on the CC stream
    (benchmarks: AG@64KB 5.2us vs AR@64KB 11.3us).
  * x1 transposes in bf16: fp32 PE transposes are 2-pass (LOW_HIGH,
    ~322ns measured) and burned 10.3us/rep of PE time.  Casting x1 to
    bf16 on DVE first makes them single-pass.
  * M is computed as (u*winv/N)^T @ x1 @ gc_w: the inner [P,D]
    intermediate Q contracts over the node dim directly from the
    node-major x1 DMA layout, replacing the h=x1@gcw (32 matmuls) +
    u^T v (8 matmuls) stages with 8 N=512 matmuls + 4 transposes + 4
    N=128 matmuls.
  * Software pipelining: the tail of rep i (AG read-back + final
    matmuls + store) is emitted after the head of rep i+DEFER with
    buffer rotation, hiding the collective latency behind other reps'
    compute.  For nreps=1 (the correctness path) order degenerates to
    head;tail — still correct.
  * AllReduce/AllGather into NON-Shared DRAM buffers (Shared-space DMA
    read-back measured ~140us/64KB on this fabric).

`nreps` unrolls the computation inside one NEFF; exec_ns is the slope
of wall(nreps), amortizing the (tens of ms) axon dispatch overhead.
"""

import os
import sys

import numpy as np

for _p in ("/opt/trn_rl_repo", "/opt/pypackages"):
    if os.path.isdir(_p) and _p not in sys.path:
        sys.path.append(_p)

import concourse.bass as bass
import concourse.mybir as mybir
from concourse import bacc
import concourse.tile as tile
from concourse.bass_utils import run_bass_kernel_spmd
from concourse.masks import make_identity

F32 = mybir.dt.float32
BF16 = mybir.dt.bfloat16
AF = mybir.ActivationFunctionType
ALU = mybir.AluOpType

N, D, P, H = 8192, 512, 128, 128
NCORES = 8
R = N // NCORES          # rows per core (1024)
NT = R // 128            # node tiles per core (8)
KD = D // 128            # contraction tiles over D (4)
NG = R // 512            # node groups of 512 (2)
INV_N = 1.0 / N
DEFER = int(os.environ.get("KERNEL_DEFER", "4"))
CC_KIND = os.environ.get("KERNEL_CC", "ag")      # "ag" | "ar"
CC_DT = BF16 if os.environ.get("KERNEL_CCDT", "f32") == "bf16" else F32


def sl(i, w=128):
    return slice(i * w, (i + 1) * w)


def build_bass(nreps: int = 1, no_cc: bool = False):
    nc = bacc.Bacc("TRN2", target_bir_lowering=False, num_devices=NCORES)

    x1 = nc.dram_tensor("x1", [R, D], F32, kind="ExternalInput")
    w1 = nc.dram_tensor("w1", [D, D], F32, kind="ExternalInput")
    w2 = nc.dram_tensor("w2", [D, P], F32, kind="ExternalInput")
    gcw = nc.dram_tensor("gcw", [D, H], F32, kind="ExternalInput")
    gcb = nc.dram_tensor("gcb", [H], F32, kind="ExternalInput")
    out = nc.dram_tensor("out", [R, H], F32, kind="ExternalOutput")

    with tile.TileContext(nc) as tc:
        with (
            tc.tile_pool(name="cpool", bufs=1) as cpool,
            tc.tile_pool(name="xload", bufs=8) as xload,
            tc.tile_pool(name="xb", bufs=3) as xbpool,
            tc.tile_pool(name="big", bufs=2) as bpool,
            tc.tile_pool(name="ut3", bufs=DEFER + 1) as ut3,
            tc.tile_pool(name="small", bufs=2) as spool,
            tc.tile_pool(name="w3", bufs=DEFER + 1) as w3,
            tc.tile_pool(name="opool", bufs=3) as opool,
            tc.tile_pool(name="mall", bufs=2) as mallp,
            tc.tile_pool(name="ptr", bufs=2, space="PSUM") as ptr,
            tc.tile_pool(name="pmm", bufs=3, space="PSUM") as pmm,
            tc.tile_pool(name="pM", bufs=1, space="PSUM") as pMp,
            tc.tile_pool(name="pfin", bufs=2, space="PSUM") as pfin,
            tc.tile_pool(name="dram", bufs=DEFER + 1, space="DRAM") as dram,
        ):
            # ---- constants / weights (loaded once, converted to bf16) ----
            identb = cpool.tile([128, 128], BF16, name="identb")
            make_identity(nc, identb)

            w1b, w2b, gcwb = [], [], []
            for kd in range(KD):
                t = xload.tile([128, D], F32, name="wload", tag="wload")
                nc.sync.dma_start(out=t, in_=w1[sl(kd), :])
                b = cpool.tile([128, D], BF16, name=f"w1b_{kd}")
                nc.vector.tensor_copy(b, t)
                w1b.append(b)
            for kd in range(KD):
                t = xload.tile([128, P], F32, name="wload", tag="wload")
                nc.sync.dma_start(out=t, in_=w2[sl(kd), :])
                b = cpool.tile([128, P], BF16, name=f"w2b_{kd}")
                nc.vector.tensor_copy(b, t)
                w2b.append(b)
            for kd in range(KD):
                t = xload.tile([128, H], F32, name="wload", tag="wload")
                nc.sync.dma_start(out=t, in_=gcw[sl(kd), :])
                b = cpool.tile([128, H], BF16, name=f"gcwb_{kd}")
                nc.vector.tensor_copy(b, t)
                gcwb.append(b)

            b_row = cpool.tile([1, H], F32, name="b_row")
            nc.sync.dma_start(out=b_row, in_=gcb[None, :])
            ones1 = cpool.tile([1, 128], F32, name="ones1")
            nc.vector.memset(ones1, 1.0)
            # bb[p, q] = gcb[q] for all p (partition-broadcast via K=1 matmul)
            bb = cpool.tile([128, H], F32, name="bb")
            pbb = pfin.tile([128, H], F32, name="pbb", tag="pp")
            nc.tensor.matmul(pbb, lhsT=ones1, rhs=b_row, start=True, stop=True)
            nc.scalar.copy(bb, pbb)

            consts = (identb, w1b, w2b, gcwb, bb)
            pools = (xload, xbpool, bpool, ut3, spool, w3, opool, mallp,
                     ptr, pmm, pMp, pfin, dram)

            inflight = {}
            for rep in range(nreps):
                inflight[rep] = _emit_head(nc, x1, pools, consts, no_cc)
                t = rep - DEFER
                if t >= 0:
                    _emit_tail(nc, out, pools, consts, inflight.pop(t), no_cc)
            for t in sorted(inflight):
                _emit_tail(nc, out, pools, consts, inflight.pop(t), no_cc)

    nc.compile()
    return nc


def _emit_head(nc, x1, pools, consts, no_cc):
    (xload, xbpool, bpool, ut3, spool, w3, opool, mallp,
     ptr, pmm, pMp, pfin, dram) = pools
    (identb, w1b, w2b, gcwb, bb) = consts

    x1T = [bpool.tile([128, R], BF16, name=f"x1T_{kd}", tag=f"x1T_{kd}")
           for kd in range(KD)]
    Bt = [bpool.tile([128, R], BF16, name=f"Bt_{mf}", tag=f"Bt_{mf}")
          for mf in range(KD)]
    ut = ut3.tile([128, R], BF16, name="ut", tag="ut")
    u4 = [bpool.tile([128, 512], BF16, name=f"u4_{g}", tag=f"u4_{g}")
          for g in range(NG)]
    u4s = [bpool.tile([128, 512], BF16, name=f"u4s_{g}", tag=f"u4s_{g}")
           for g in range(NG)]
    winv = w3.tile([128, NT], F32, name="winv", tag="winv")
    ssq = spool.tile([128, NT], F32, name="ssq", tag="ssq")
    xrb = [xbpool.tile([128, D], BF16, name=f"xrb_{m}", tag=f"xrb_{m}")
           for m in range(NT)]

    # ---- stage 1: load x1, cast to bf16, transpose (single-pass bf16) --
    for g in range(NG):
        gs = sl(g, 512)
        for j in range(4):
            m = 4 * g + j
            xr = xload.tile([128, D], F32, name="xr", tag="xr")
            nc.sync.dma_start(out=xr, in_=x1[sl(m), :])
            nc.vector.tensor_copy(xrb[m], xr)
        for kd in range(KD):
            ptx = ptr.tile([128, 512], BF16, name="ptx", tag="tr")
            for j in range(4):
                nc.tensor.transpose(ptx[:, sl(j)], xrb[4 * g + j][:, sl(kd)],
                                    identb)
            nc.vector.tensor_copy(x1T[kd][:, gs], ptx)

    # ---- stage 2: projector chains -------------------------------------
    for g in range(NG):
        gs = sl(g, 512)
        for mf in range(KD):
            pb = pmm.tile([128, 512], F32, name="pb", tag="mm")
            for kd in range(KD):
                nc.tensor.matmul(
                    pb,
                    lhsT=w1b[kd][:, sl(mf)],
                    rhs=x1T[kd][:, gs],
                    start=(kd == 0),
                    stop=(kd == KD - 1),
                )
            nc.scalar.activation(Bt[mf][:, gs], pb, AF.Relu)
    for g in range(NG):
        gs = sl(g, 512)
        pu = pmm.tile([128, 512], F32, name="pu", tag="mm")
        for mf in range(KD):
            nc.tensor.matmul(
                pu,
                lhsT=w2b[mf],
                rhs=Bt[mf][:, gs],
                start=(mf == 0),
                stop=(mf == KD - 1),
            )
        nc.vector.tensor_copy(ut[:, gs], pu)

    # ---- stage 3: u -> node-major + row norms + scale by winv/N --------
    for g in range(NG):
        ptu = ptr.tile([128, 512], BF16, name="ptu", tag="tr")
        for j in range(4):
            nc.tensor.transpose(ptu[:, sl(j)], ut[:, sl(4 * g + j)], identb)
        nc.vector.tensor_copy(u4[g], ptu)
        for j in range(4):
            m = 4 * g + j
            sq = spool.tile([128, 128], F32, name="sq", tag="sq")
            nc.scalar.activation(sq, ptu[:, sl(j)], AF.Square,
                                 accum_out=ssq[:, m:m + 1])
    wv = spool.tile([128, NT], F32, name="wv", tag="wv")
    nc.scalar.activation(wv, ssq, AF.Sqrt)
    nc.vector.reciprocal(winv, wv)
    for g in range(NG):
        for j in range(4):
            m = 4 * g + j
            nc.vector.tensor_scalar(
                u4s[g][:, sl(j)], u4[g][:, sl(j)], winv[:, m:m + 1], INV_N,
                op0=ALU.mult, op1=ALU.mult,
            )

    # ---- stage 4: Q = (u*winv/N)^T @ x1 [P,D]; Mp = Q @ gcw [P,H] ------
    pQ = pmm.tile([128, 512], F32, name="pQ", tag="mm")
    for m in range(NT):
        g, j = divmod(m, 4)
        nc.tensor.matmul(
            pQ,
            lhsT=u4s[g][:, sl(j)],
            rhs=xrb[m],
            start=(m == 0),
            stop=(m == NT - 1),
        )
    Qsb = spool.tile([128, 512], BF16, name="Qsb", tag="Qsb")
    nc.vector.tensor_copy(Qsb, pQ)
    ptq = ptr.tile([128, 512], BF16, name="ptq", tag="tr")
    for kd in range(KD):
        nc.tensor.transpose(ptq[:, sl(kd)], Qsb[:, sl(kd)], identb)
    QT = spool.tile([128, 512], BF16, name="QT", tag="QT")
    nc.vector.tensor_copy(QT, ptq)
    pM = pMp.tile([128, H], F32, name="pM", tag="pM")
    for kd in range(KD):
        nc.tensor.matmul(
            pM,
            lhsT=QT[:, sl(kd)],
            rhs=gcwb[kd],
            start=(kd == 0),
            stop=(kd == KD - 1),
        )
    Msb = w3.tile([128, H], CC_DT, name="Msb", tag="Msb")
    nc.scalar.copy(Msb, pM)

    # ---- stage 5: partial-M exchange kickoff ---------------------------
    cc_out = None
    if not no_cc:
        cc_in = dram.tile([128, H], CC_DT, name="cc_in", tag="cc_in")
        nc.sync.dma_start(out=cc_in, in_=Msb)
        if CC_KIND == "ag":
            cc_out = dram.tile([128 * NCORES, H], CC_DT, name="cc_out",
                               tag="cc_out")
            nc.gpsimd.collective_compute(
                "AllGather",
                ALU.bypass,
                replica_groups=[list(range(NCORES))],
                ins=[cc_in[:, :]],
                outs=[cc_out[:, :]],
            )
        else:
            cc_out = dram.tile([128, H], CC_DT, name="cc_out", tag="cc_out")
            nc.gpsimd.collective_compute(
                "AllReduce",
                ALU.add,
                replica_groups=[list(range(NCORES))],
                ins=[cc_in[:, :]],
                outs=[cc_out[:, :]],
            )
    return (cc_out, Msb, ut, winv)


def _emit_tail(nc, out, pools, consts, state, no_cc):
    (xload, xbpool, bpool, ut3, spool, w3, opool, mallp,
     ptr, pmm, pMp, pfin, dram) = pools
    (identb, w1b, w2b, gcwb, bb) = consts
    (cc_out, Msb, ut, winv) = state

    Mb = spool.tile([128, H], BF16, name="Mb", tag="Mb")
    if no_cc:  # timing-only variant: skip the collective (math wrong)
        nc.vector.tensor_copy(Mb, Msb)
    elif CC_KIND == "ag":
        Mall = mallp.tile([128, NCORES * H], CC_DT, name="Mall", tag="Mall")
        for g in range(NCORES):
            nc.sync.dma_start(out=Mall[:, sl(g)], in_=cc_out[sl(g), :])
        # tree-sum the 8 partials on DVE (f32 accumulation)
        t4 = [spool.tile([128, H], F32, name=f"t4_{i}", tag=f"t4_{i}")
              for i in range(4)]
        for i in range(4):
            nc.vector.tensor_tensor(t4[i], Mall[:, sl(2 * i)],
                                    Mall[:, sl(2 * i + 1)], ALU.add)
        t2 = [spool.tile([128, H], F32, name=f"t2_{i}", tag=f"t2_{i}")
              for i in range(2)]
        for i in range(2):
            nc.vector.tensor_tensor(t2[i], t4[2 * i], t4[2 * i + 1], ALU.add)
        nc.vector.tensor_tensor(Mb, t2[0], t2[1], ALU.add)
    else:
        Mred = spool.tile([128, H], CC_DT, name="Mred", tag="Mred")
        nc.sync.dma_start(out=Mred, in_=cc_out[:, :])
        nc.vector.tensor_copy(Mb, Mred)

    # ---- out = (u @ M) * winv + bb -------------------------------------
    for m in range(NT):
        pp = pfin.tile([128, H], F32, name="pp", tag="pp")
        nc.tensor.matmul(pp, lhsT=ut[:, sl(m)], rhs=Mb, start=True, stop=True)
        ob = opool.tile([128, H], F32, name="ob", tag="ob")
        nc.vector.scalar_tensor_tensor(
            ob, pp, winv[:, m:m + 1], bb, op0=ALU.mult, op1=ALU.add
        )
        nc.sync.dma_start(out=out[sl(m), :], in_=ob)


_NCS = {}
LAST_RESULTS = None
_RUNNERS = {}


def _get_nc(nreps: int = 1):
    key = (nreps, os.environ.get("KERNEL_NO_CC") == "1")
    if key not in _NCS:
        _NCS[key] = build_bass(nreps, no_cc=key[1])
    return _NCS[key]


def _split_in_maps(inputs):
    x1 = np.ascontiguousarray(np.asarray(inputs["x1"], dtype=np.float32))
    w1 = np.ascontiguousarray(np.asarray(inputs["proj_w1"], dtype=np.float32))
    w2 = np.ascontiguousarray(np.asarray(inputs["proj_w2"], dtype=np.float32))
    gcw = np.ascontiguousarray(np.asarray(inputs["gc_w"], dtype=np.float32))
    gcb = np.ascontiguousarray(np.asarray(inputs["gc_b"], dtype=np.float32))
    return [
        {
            "x1": np.ascontiguousarray(x1[c * R:(c + 1) * R]),
            "w1": w1,
            "w2": w2,
            "gcw": gcw,
            "gcb": gcb,
        }
        for c in range(NCORES)
    ]


def kernel(**inputs) -> np.ndarray:
    global LAST_RESULTS
    res = run_bass_kernel_spmd(
        _get_nc(1), _split_in_maps(inputs), core_ids=list(range(NCORES))
    )
    LAST_RESULTS = res
    return np.concatenate([res.results[c]["out"] for c in range(NCORES)], axis=0)


# ---------------------------------------------------------------------------
# Timing path: the nreps-unrolled NEFF amortizes the (tens of ms) axon
# dispatch overhead; per-exec time = slope between two nreps points.
# ---------------------------------------------------------------------------

def _make_runner(nreps: int):
    if nreps in _RUNNERS:
        return _RUNNERS[nreps]
    import jax
    import concourse.mybir as mybir_
    from concourse.bass2jax import (
        _bass_exec_p,
        install_neuronx_cc_hook,
        partition_id_tensor,
    )
    from jax.experimental.shard_map import shard_map
    from jax.sharding import Mesh, PartitionSpec

    nc = _get_nc(nreps)
    install_neuronx_cc_hook()
    partition_name = (
        nc.partition_id_tensor.name if nc.partition_id_tensor else None
    )

    in_names, out_names, out_avals = [], [], []
    for alloc in nc.m.functions[0].allocations:
        if not isinstance(alloc, mybir_.MemoryLocationSet):
            continue
        name = alloc.memorylocations[0].name
        if alloc.kind == "ExternalInput":
            if name != partition_name:
                in_names.append(name)
        elif alloc.kind == "ExternalOutput":
            out_names.append(name)
            out_avals.append(
                jax.core.ShapedArray(
                    tuple(alloc.tensor_shape), mybir_.dt.np(alloc.dtype)
                )
            )
    n_params = len(in_names)
    all_names = in_names + out_names
    if partition_name is not None:
        all_names = all_names + [partition_name]

    def _body(*args):
        operands = list(args)
        if partition_name is not None:
            operands.append(partition_id_tensor())
        outs = _bass_exec_p.bind(
            *operands,
            out_avals=tuple(out_avals),
            in_names=tuple(all_names),
            out_names=tuple(out_names),
            lowering_input_output_aliases=(),
            sim_require_finite=True,
            sim_require_nnan=True,
            nc=nc,
        )
        return tuple(outs)

    devices = jax.devices()[:NCORES]
    mesh = Mesh(np.asarray(devices), ("core",))
    nin = n_params + len(out_names)
    sharded = jax.jit(
        shard_map(
            _body,
            mesh=mesh,
            in_specs=(PartitionSpec("core"),) * nin,
            out_specs=(PartitionSpec("core"),) * len(out_names),
            check_rep=False,
        ),
        keep_unused=True,
    )
    meta = (in_names, out_names, out_avals, n_params)
    _RUNNERS[nreps] = (sharded, meta)
    return _RUNNERS[nreps]


def _prep_args(inputs, nreps: int):
    import jax

    sharded, meta = _make_runner(nreps)
    in_names, out_names, out_avals, n_params = meta
    in_maps = _split_in_maps(inputs)
    concat_in = [
        np.concatenate([np.asarray(in_maps[c][n]) for c in range(NCORES)], axis=0)
        for n in in_names
    ]
    concat_zeros = [
        np.zeros((NCORES * a.shape[0], *a.shape[1:]), a.dtype) for a in out_avals
    ]
    args = [jax.device_put(a) for a in concat_in + concat_zeros]
    for a in args:
        a.block_until_ready()
    return sharded, args


def _timed_call(sharded, args):
    import time

    t0 = time.perf_counter()
    outs = sharded(*args)
    for o in outs:
        o.block_until_ready()
    return time.perf_counter() - t0, outs


def run_repeated(inputs, nreps: int, iters: int = 6):
    """Run the nreps-unrolled NEFF; returns (out_core0, min_wall_seconds)."""
    sharded, args = _prep_args(inputs, nreps)
    _timed_call(sharded, args)  # warmup/compile
    times = []
    outs = None
    for _ in range(iters):
        t, outs = _timed_call(sharded, args)
        times.append(t)
    return np.asarray(outs[0]), min(times)


def measure_exec_ns(inputs, k1=8, k2=104, rounds=25):
    """Amortized per-exec device time via interleaved two-point slope.

    Axon dispatch overhead drifts by tens of ms over minutes, so the k1
    and k2 NEFFs are compiled+warmed FIRST, then timed strictly
    interleaved so both sample the same overhead distribution; the
    min-over-rounds of each cancels the (additive, positive) noise.
    """
    import statistics

    s1, a1 = _prep_args(inputs, k1)
    s2, a2 = _prep_args(inputs, k2)
    for _ in range(3):  # warm both (compile, caches, power state)
        _timed_call(s1, a1)
        _timed_call(s2, a2)
    t1s, t2s = [], []
    outs2 = None
    for _ in range(rounds):
        t1, _o = _timed_call(s1, a1)
        t2, outs2 = _timed_call(s2, a2)
        t1s.append(t1)
        t2s.append(t2)
    dk = k2 - k1
    slope_min = (min(t2s) - min(t1s)) / dk
    pair = sorted((b - a) / dk for a, b in zip(t1s, t2s))
    slope_pairmed = statistics.median(pair)
    print(
        f"timing diag: min-min={slope_min*1e9:.0f}ns "
        f"pair-med={slope_pairmed*1e9:.0f}ns "
        f"pair-q25={pair[len(pair)//4]*1e9:.0f}ns "
        f"pair-q75={pair[3*len(pair)//4]*1e9:.0f}ns "
        f"t1(min/med)={min(t1s)*1e3:.2f}/{statistics.median(t1s)*1e3:.2f}ms "
        f"t2(min/med)={min(t2s)*1e3:.2f}/{statistics.median(t2s)*1e3:.2f}ms"
    )
    per_exec = slope_min
    return per_exec * 1e9, np.asarray(outs2[0]), min(t1s), min(t2s)
